# revision 30
# baseline (speedup 1.0000x reference)
"""Trainium2 Bass kernel for a dense transformer block (pre-norm, causal MHA + GELU FFN).

Distribution over 8 NeuronCores:
  Phase 1 (head tensor-parallel): every core holds the full activations and
  computes Q/K/V projections, causal attention and the per-head attention
  output for its 2 of 16 heads.  One AllToAll per batch exchanges the
  attention-output slices so each core ends up with all 2048 head-dims for
  1/8 of the tokens, with no on-device transposes.
  Phase 2 (token-parallel): each core does out-projection + residual, rmsnorm
  and the full FFN for its 512 tokens, streaming the FFN weights from HBM.

Precision strategy (rel-err budget 2e-2):
  - x is quantized to fp8(e4m3) on the host in the exact per-block SBUF
    layout; the phase-1 rmsnorm scale (host-computed, depends only on x) is
    folded into the Q/K/V PSUM->SBUF copies instead of pre-scaling x.
  - QKV projections, softmax numerator/denominator, attn@V and the output
    projection run as fp8 DoubleRow matmuls; logits stay float32r.
  - The FFN runs in bf16; fp8 there would blow the error budget.
  - Residual stream stays fp32 end-to-end.

Scheduling:
  - Collectives issue on the gpsimd queue.  gpsimd elementwise work is
    rerouted to DVE in the windows where an AllToAll is in flight, and the
    oT gathers issue from gpsimd (on SP they would head-of-line block and
    starve every later DMA issue).
  - QKV weights + mask + rscP are loaded once, resident across reps.
  - Phase 2 runs out-projection/residual/rmsnorm per batch, plus the first
    F0 FFN1 chunks on batch 0 only, so batch 1's AllToAll is hidden behind
    real work; the w1 chunks 0..F0 are re-streamed later for batch 1.
  - The a2a payload is partition-major per dest so the receive-side gather
    reads contiguous 512B lines; wo loads are paired and xres loads grouped
    by 4 (DMA issue slots on the sync engine are a measurable resource).
  - A merged single AllToAll measured ~60us slower than two per-batch ones
    (collective cost scales with payload and loses the batch-0 overlap).
"""

import numpy as np
import ml_dtypes

# Model dims (hardcoded per the problem spec)
DIM = 2048
T = 2048
B = 2
H = 16
HD = 128
FF = 8192
EPS = 1e-5
SCALE = HD ** -0.5

NCORES = 8
P = 128
HPC = H // NCORES      # heads per core = 2
HDC = HPC * HD         # head dims per core = 256
DCH = DIM // P         # 16 chunks of the model dim
QB = 512               # query block
NQB = T // QB          # 4 query blocks per batch
NB = B * NQB           # 8 blocks total
ASH = T // NCORES      # tokens per A2A shard = 256
TPC = B * ASH          # tokens per core in phase 2 = 512
FCH = FF // P          # 64 ff chunks
KP = DCH // 2          # fp8 DoubleRow contraction pairs over the model dim
F0 = 24                # FFN1 chunks run batch-0-first to hide b1's AllToAll

E4NP = ml_dtypes.float8_e4m3
BFNP = ml_dtypes.bfloat16

_CACHE = {}


def _build_program(reps=1, collectives=True, cc_mode="both"):
    import concourse.mybir as mybir
    import concourse.tile as tile
    from concourse import bacc

    dt = mybir.dt
    f32 = dt.float32
    f32r = dt.float32r
    bf16 = dt.bfloat16
    f8 = dt.float8e4
    AF = mybir.ActivationFunctionType
    ALU = mybir.AluOpType
    DR = mybir.MatmulPerfMode.DoubleRow

    nc = bacc.Bacc("TRN2", target_bir_lowering=False, debug=False,
                   num_devices=NCORES)

    # ---- I/O ----
    x8_d = nc.dram_tensor("x8T", [P, NB, DCH, QB], f8, kind="ExternalInput")
    rsc_d = nc.dram_tensor("rscT", [P, B * T], f32, kind="ExternalInput")
    rscP_d = nc.dram_tensor("rscP", [P, B * T // P], f32,
                            kind="ExternalInput")
    xres_d = nc.dram_tensor("xresT", [DIM, TPC], f32, kind="ExternalInput")
    wq_d = nc.dram_tensor("wq_l", [P, DCH, HDC], f8, kind="ExternalInput")
    wk_d = nc.dram_tensor("wk_l", [P, DCH, HDC], f8, kind="ExternalInput")
    wv_d = nc.dram_tensor("wv_l", [P, DCH, HDC], f8, kind="ExternalInput")
    wo_d = nc.dram_tensor("wo_s", [DCH, P, DCH, P], f8, kind="ExternalInput")
    w1_d = nc.dram_tensor("w1_s", [FCH, P, DCH, P], bf16,
                          kind="ExternalInput")
    w2_d = nc.dram_tensor("w2_s", [DCH, P, FCH, P], bf16,
                          kind="ExternalInput")
    mask_d = nc.dram_tensor("mask_l", [P, QB // P, QB], f8,
                            kind="ExternalInput")
    out_d = nc.dram_tensor("outT", [DIM, TPC], f32, kind="ExternalOutput")

    # ---- internal DRAM (a2a payload, partition-major per dest so the
    # receive-side gather reads contiguous 512B lines per partition).
    # One exchange per batch: batch 0's overlaps batch 1's attention, and
    # a merged single exchange measured slower (cost scales with payload).
    a2a_in = [nc.dram_tensor(f"a2a_in{b}", [NCORES, P, HPC, ASH], f8)
              for b in range(B)]
    a2a_out = [nc.dram_tensor(f"a2a_out{b}", [NCORES, P, HPC, ASH], f8)
               for b in range(B)]

    xres_r = xres_d.ap().rearrange("(k p) t -> p k t", p=P)
    out_r = out_d.ap().rearrange("(k p) t -> p k t", p=P)
    wo_pair = wo_d.ap().rearrange("(mm two) p k n -> mm p two k n", two=2)

    with tile.TileContext(nc) as tc:
        from contextlib import ExitStack
        with ExitStack() as ctx:
            consts = ctx.enter_context(tc.tile_pool(name="consts", bufs=1))
            ones_f = consts.tile([P, P], f32)
            nc.vector.memset(ones_f, 1.0)
            ones_r = consts.tile([P, P], f32r)
            nc.vector.tensor_copy(ones_r, ones_f)
            ones8 = consts.tile([P, 2, P], f8)
            nc.vector.tensor_copy(ones8[:, 0, :], ones_f)
            nc.vector.tensor_copy(ones8[:, 1, :], ones_f)

            # persistent weights: loaded once, resident across reps
            wq_sb = consts.tile([P, DCH, HDC], f8)
            wk_sb = consts.tile([P, DCH, HDC], f8)
            wv_sb = consts.tile([P, DCH, HDC], f8)
            mask_sb = consts.tile([P, QB // P, QB], f8)
            rscP_sb = consts.tile([P, B * T // P], f32)
            nc.sync.dma_start(wq_sb, wq_d.ap())
            nc.sync.dma_start(wk_sb, wk_d.ap())
            nc.sync.dma_start(wv_sb, wv_d.ap())
            nc.sync.dma_start(mask_sb, mask_d.ap())
            nc.sync.dma_start(rscP_sb, rscP_d.ap())

            for _rep in range(reps):
                with ExitStack() as rep:
                    # streams + oT live outside the phase scopes so their
                    # DMAs can issue early (overlap previous phase / rep)
                    strm = rep.enter_context(tc.tile_pool(name="strm",
                                                          bufs=1))
                    oT = strm.tile([P, DCH, TPC], f8, tag="oT")
                    wstream = rep.enter_context(
                        tc.tile_pool(name="wstr", bufs=3))
                    w2stream = rep.enter_context(
                        tc.tile_pool(name="w2str", bufs=2))
                    xrstream = rep.enter_context(
                        tc.tile_pool(name="xrstr", bufs=2))

                    # ============ PHASE 1 ============
                    with ExitStack() as p1:
                        x8_pool = p1.enter_context(
                            tc.tile_pool(name="x8", bufs=2))
                        sm_pool = p1.enter_context(
                            tc.tile_pool(name="p1sm", bufs=2))
                        qkv_out = p1.enter_context(
                            tc.tile_pool(name="qkvo", bufs=1))
                        q_pool = p1.enter_context(
                            tc.tile_pool(name="qp", bufs=2))
                        exp_pool = p1.enter_context(
                            tc.tile_pool(name="expp", bufs=3))
                        o_pool = p1.enter_context(
                            tc.tile_pool(name="op", bufs=2))

                        ps1 = p1.enter_context(
                            tc.tile_pool(name="ps1", bufs=1, space="PSUM"))

                        def fetch_block(i):
                            """DMA block i's fp8 x (host-prequantized, laid
                            out contiguously per partition) + its rsc row."""
                            xb8 = x8_pool.tile([P, DCH, QB], f8, tag="x8")
                            nc.sync.dma_start(xb8, x8_d.ap()[:, i])
                            rsc = sm_pool.tile([P, QB], f32, tag="rsc")
                            tok0 = i * QB
                            nc.sync.dma_start(rsc,
                                              rsc_d.ap()[:, tok0:tok0 + QB])
                            return xb8, rsc

                        nxt = fetch_block(0)
                        kT = vn = None
                        for i in range(NB):
                            b, blk = divmod(i, NQB)
                            if blk == 0:
                                kT = qkv_out.tile([P, HPC, T], f32r,
                                                  tag="kT")
                                vn = qkv_out.tile([P, T // P, HDC], f8,
                                                  tag="vn")
                            xb8, rsc_sb = nxt
                            if i + 1 < NB:
                                nxt = fetch_block(i + 1)
                            # while the b0 AllToAll is in flight (blocks
                            # 4-5) gpsimd must stay free; DVE covers it
                            gp_free = collectives and i in (4, 5)
                            veng = nc.vector if gp_free else nc.gpsimd

                            # Q^T, K^T for this block: [hd 128, tok 512],
                            # rmsnorm scale folded into the PSUM->SBUF copy
                            qloc = q_pool.tile([P, HPC, QB], f32r,
                                               tag="qloc")
                            for m in range(HPC):
                                ps = ps1.tile([P, QB], f32, tag="psqk",
                                              bufs=3)
                                for kp in range(KP):
                                    nc.tensor.matmul(
                                        ps,
                                        wq_sb[:, 2 * kp:2 * kp + 2,
                                              m * P:(m + 1) * P],
                                        xb8[:, 2 * kp:2 * kp + 2, :],
                                        start=(kp == 0), stop=(kp == KP - 1),
                                        perf_mode=DR)
                                nc.vector.tensor_mul(qloc[:, m, :], ps,
                                                     rsc_sb)
                            for m in range(HPC):
                                ps = ps1.tile([P, QB], f32, tag="psqk",
                                              bufs=3)
                                for kp in range(KP):
                                    nc.tensor.matmul(
                                        ps,
                                        wk_sb[:, 2 * kp:2 * kp + 2,
                                              m * P:(m + 1) * P],
                                        xb8[:, 2 * kp:2 * kp + 2, :],
                                        start=(kp == 0), stop=(kp == KP - 1),
                                        perf_mode=DR)
                                nc.vector.tensor_mul(
                                    kT[:, m, blk * QB:(blk + 1) * QB], ps,
                                    rsc_sb)
                            # V natural: [tok 128, hd 256]; scale is
                            # per-partition here (rscP)
                            for ts in range(QB // P):
                                psf = ps1.tile([P, QB], f32, tag="psqk",
                                               bufs=3, name="psv")
                                ps = psf[:, :HDC]
                                for kp in range(KP):
                                    nc.tensor.matmul(
                                        ps,
                                        xb8[:, 2 * kp:2 * kp + 2,
                                            ts * P:(ts + 1) * P],
                                        wv_sb[:, 2 * kp:2 * kp + 2, :],
                                        start=(kp == 0), stop=(kp == KP - 1),
                                        perf_mode=DR)
                                j = i * (QB // P) + ts
                                nc.scalar.activation(
                                    vn[:, blk * 4 + ts, :], ps, AF.Copy,
                                    scale=rscP_sb[:, j:j + 1])

                            # ---- attention for q-block = blk (causal) ----
                            # Two passes per head: (1) all logits+exp+mask,
                            # (2) all denominator/PV matmuls.
                            qb = blk
                            nkc = (qb + 1) * (QB // P)
                            for h in range(HPC):
                                etps = []
                                for kp in range(nkc // 2):
                                    etp = exp_pool.tile([P, 2, QB], f8,
                                                        tag="et", bufs=16)
                                    etps.append(etp)
                                    for sub in range(2):
                                        kc = 2 * kp + sub
                                        psl = ps1.tile([P, QB], f32,
                                                       tag="psl", bufs=3)
                                        nc.tensor.matmul(
                                            psl,
                                            kT[:, h, kc * P:(kc + 1) * P],
                                            qloc[:, h, :],
                                            start=True, stop=True)
                                        nc.scalar.activation(
                                            etp[:, sub, :], psl, AF.Exp,
                                            scale=SCALE)
                                        rel = kc - qb * (QB // P)
                                        if rel >= 0:
                                            # always DVE: gpsimd may be
                                            # blocked by an in-flight a2a
                                            nc.vector.tensor_mul(
                                                etp[:, sub, :],
                                                etp[:, sub, :],
                                                mask_sb[:, rel, :])
                                psd = ps1.tile([P, QB], f32, tag="psden",
                                               bufs=1)
                                pso = ps1.tile([P, QB], f32, tag="pso",
                                               bufs=1)
                                for kp in range(nkc // 2):
                                    etp = etps[kp]
                                    nc.tensor.matmul(
                                        psd, ones8, etp,
                                        start=(kp == 0),
                                        stop=(kp == nkc // 2 - 1),
                                        perf_mode=DR)
                                    nc.tensor.matmul(
                                        pso,
                                        vn[:, 2 * kp:2 * kp + 2,
                                           h * P:(h + 1) * P],
                                        etp,
                                        start=(kp == 0),
                                        stop=(kp == nkc // 2 - 1),
                                        perf_mode=DR)
                                rden = sm_pool.tile([P, QB], f32,
                                                    tag="rden")
                                nc.vector.reciprocal(rden, psd)
                                osb = o_pool.tile([P, QB], f8, tag="osb")
                                nc.vector.tensor_mul(osb, pso, rden)
                                # a2a payload: [dest, p, head, tok]
                                for half in range(2):
                                    d = qb * 2 + half
                                    nc.sync.dma_start(
                                        a2a_in[b].ap()[d, :, h, :],
                                        osb[:,
                                            half * ASH:(half + 1) * ASH])

                            if blk == NQB - 1:
                                do_cc = collectives and (
                                    cc_mode == "both"
                                    or (cc_mode == "b0" and b == 0))
                                if do_cc:
                                    nc.gpsimd.collective_compute(
                                        "AllToAll",
                                        mybir.AluOpType.bypass,
                                        replica_groups=[list(range(NCORES))],
                                        ins=[a2a_in[b].ap()],
                                        outs=[a2a_out[b].ap()],
                                    )
                                # gather this batch's a2a segments straight
                                # into oT [hd-chunk, tok].  Issued on the
                                # gpsimd queue: on SP it would head-of-line
                                # block (waiting on the collective) and
                                # starve every later DMA issue.
                                for s in range(NCORES):
                                    nc.gpsimd.dma_start(
                                        oT[:, 2 * s:2 * s + 2,
                                           b * ASH:(b + 1) * ASH],
                                        a2a_out[b].ap()[s])

                    # ============ PHASE 2 ============
                    with ExitStack() as p2:
                        big = p2.enter_context(tc.tile_pool(name="p2big",
                                                            bufs=1))
                        sm2 = p2.enter_context(tc.tile_pool(name="p2sm",
                                                            bufs=1))
                        ps2 = p2.enter_context(
                            tc.tile_pool(name="ps2", bufs=1, space="PSUM"))

                        x2T = big.tile([P, DCH, TPC], f32, tag="x2")
                        hT = big.tile([P, DCH, TPC], bf16, tag="hT")
                        uu = big.tile([P, FCH, TPC], bf16, tag="u")
                        rsc2 = [None, None]

                        # out-projection + residual + rmsnorm, one batch at
                        # a time: batch 0 starts immediately (its a2a landed
                        # mid-phase-1), hiding batch 1's a2a.  Scalar squares
                        # + DVE adds shorten the chain to hT that gates the
                        # FFN1 cover work.
                        for b in range(B):
                            tk = slice(b * ASH, (b + 1) * ASH)
                            acc = sm2.tile([P, ASH], f32r, tag="accD",
                                           bufs=2)
                            accG = sm2.tile([P, ASH], f32r, tag="accG",
                                            bufs=1)
                            for m in range(DCH):
                                if m % 2 == 0:
                                    wo_sb = wstream.tile([P, 2, DCH, P], f8,
                                                         tag="wo", bufs=2)
                                    nc.sync.dma_start(wo_sb,
                                                      wo_pair[m // 2])
                                if m % 4 == 0:
                                    xres_c = xrstream.tile(
                                        [P, 4, ASH], f32, tag="xresc",
                                        bufs=2)
                                    nc.sync.dma_start(
                                        xres_c, xres_r[:, m:m + 4, tk])
                                ps = ps2.tile([P, ASH], f32, tag="ps2w",
                                              bufs=3)
                                for kp in range(KP):
                                    nc.tensor.matmul(
                                        ps,
                                        wo_sb[:, m % 2, 2 * kp:2 * kp + 2,
                                              :],
                                        oT[:, 2 * kp:2 * kp + 2, tk],
                                        start=(kp == 0),
                                        stop=(kp == KP - 1),
                                        perf_mode=DR)
                                nc.vector.tensor_add(x2T[:, m, tk], ps,
                                                     xres_c[:, m % 4, :])
                                # batch 0 runs while the b1 AllToAll is in
                                # flight on gpsimd: keep gpsimd free there
                                on_gp = b == 1 and m % 4 == 3
                                seng = nc.gpsimd if on_gp else nc.vector
                                accX = accG if on_gp else acc
                                first = m == 0 or (on_gp and m == 3)
                                if first:  # first touch of acc / accG
                                    seng.tensor_mul(accX, x2T[:, m, tk],
                                                    x2T[:, m, tk])
                                else:
                                    sqm = sm2.tile([P, ASH], f32r,
                                                   tag="sqm", bufs=2)
                                    if on_gp:
                                        seng.tensor_mul(sqm, x2T[:, m, tk],
                                                        x2T[:, m, tk])
                                    else:
                                        nc.scalar.activation(
                                            sqm, x2T[:, m, tk], AF.Square)
                                    seng.tensor_add(accX, accX, sqm)

                            # rmsnorm -> hT (bf16)
                            if b == 1:
                                nc.vector.tensor_add(acc, acc, accG)
                            ps_ss = ps2.tile([P, ASH], f32, tag="ps2w",
                                             bufs=3)
                            nc.tensor.matmul(ps_ss, ones_r, acc,
                                             start=True, stop=True)
                            ms2 = sm2.tile([P, ASH], f32, tag="ms2", bufs=1)
                            nc.vector.tensor_scalar(
                                ms2, ps_ss, 1.0 / DIM, EPS,
                                mybir.AluOpType.mult, mybir.AluOpType.add)
                            rms = sm2.tile([P, ASH], f32, tag="rms2",
                                           bufs=1)
                            nc.scalar.activation(rms, ms2, AF.Sqrt)
                            rsc2[b] = sm2.tile([P, ASH], f32, tag="rsc2",
                                               bufs=2, name=f"rsc2_{b}")
                            nc.vector.reciprocal(rsc2[b], rms)
                            for k in range(DCH):
                                heng = (nc.gpsimd if b == 1 and k >= 12
                                        else nc.vector)
                                heng.tensor_mul(hT[:, k, tk],
                                                x2T[:, k, tk], rsc2[b])

                            if b == 0:
                                # FFN1 on batch 0 only for the first F0
                                # chunks: PE work covering b1's AllToAll
                                for f in range(F0):
                                    w1_sb = wstream.tile([P, DCH, P], bf16,
                                                         tag="w1b", bufs=3)
                                    nc.sync.dma_start(w1_sb, w1_d.ap()[f])
                                    psu = ps2.tile([P, TPC], f32, tag="psu",
                                                   bufs=3)
                                    psh = psu[:, :ASH]
                                    for kc in range(DCH):
                                        nc.tensor.matmul(
                                            psh, w1_sb[:, kc, :],
                                            hT[:, kc, tk],
                                            start=(kc == 0),
                                            stop=(kc == DCH - 1))
                                    nc.scalar.activation(uu[:, f, tk], psh,
                                                         AF.Gelu)

                        # FFN1 rest: both batches (512 tokens) for chunks
                        # F0.., then batch-1 catch-up on chunks 0..F0
                        for f in list(range(F0, FCH)) + list(range(F0)):
                            both = f >= F0
                            w1_sb = wstream.tile([P, DCH, P], bf16,
                                                 tag="w1b", bufs=3)
                            nc.sync.dma_start(w1_sb, w1_d.ap()[f])
                            psu = ps2.tile([P, TPC], f32, tag="psu", bufs=3)
                            sl = slice(0, TPC) if both else slice(ASH, TPC)
                            pview = psu[:, sl]
                            for kc in range(DCH):
                                nc.tensor.matmul(pview, w1_sb[:, kc, :],
                                                 hT[:, kc, sl],
                                                 start=(kc == 0),
                                                 stop=(kc == DCH - 1))
                            nc.scalar.activation(uu[:, f, sl], pview,
                                                 AF.Gelu)

                        for m in range(DCH):
                            w2_sb = w2stream.tile([P, FCH, P], bf16,
                                                  tag="w2")
                            nc.sync.dma_start(w2_sb, w2_d.ap()[m])
                            psz = ps2.tile([P, TPC], f32, tag="psz", bufs=2)
                            for fc in range(FCH):
                                nc.tensor.matmul(psz, w2_sb[:, fc, :],
                                                 uu[:, fc, :],
                                                 start=(fc == 0),
                                                 stop=(fc == FCH - 1))
                            zc = sm2.tile([P, TPC], f32, tag="zc", bufs=1)
                            nc.vector.tensor_add(zc, psz, x2T[:, m, :])
                            nc.sync.dma_start(out_r[:, m, :], zc)

    nc.compile()
    return nc


def _host_prep(x, attn_norm_w, wq, wk, wv, wo, ff_norm_w, w1, w2):
    f32 = np.float32
    xf = np.ascontiguousarray(x.reshape(B * T, DIM).T, dtype=f32)  # [D, BT]
    # host-side fp8 quantize of x in the per-block SBUF layout
    # [P, NB, DCH, QB]: partition p, block i, chunk k, token t
    x8 = (xf.reshape(DCH, P, NB, QB).transpose(1, 2, 0, 3)).astype(E4NP)
    x8 = np.ascontiguousarray(x8)
    # phase-1 rmsnorm scale (from f32 x), replicated across partitions
    rsc = 1.0 / np.sqrt((xf * xf).mean(axis=0) + EPS)  # [BT]
    rscT = np.ascontiguousarray(
        np.broadcast_to(rsc[None, :], (P, B * T))).astype(f32)
    rscP = np.ascontiguousarray(rsc.reshape(B * T // P, P).T).astype(f32)

    wq_e = (wq * attn_norm_w[None, :]).astype(f32)
    wk_e = (wk * attn_norm_w[None, :]).astype(f32)
    wv_e = (wv * attn_norm_w[None, :]).astype(f32)
    w1_e = (w1 * ff_norm_w[None, :]).astype(f32)

    wo_s = np.ascontiguousarray(
        wo.T.reshape(DCH, P, DCH, P).transpose(2, 1, 0, 3)).astype(E4NP)
    w1_s = np.ascontiguousarray(
        w1_e.T.reshape(DCH, P, FCH, P).transpose(2, 1, 0, 3)).astype(BFNP)
    w2_s = np.ascontiguousarray(
        w2.T.reshape(FCH, P, DCH, P).transpose(2, 1, 0, 3)).astype(BFNP)

    rel = np.arange(QB // P)[:, None, None] * P + np.arange(P)[None, :, None]
    masks = (rel <= np.arange(QB)[None, None, :]).astype(f32).astype(E4NP)
    mask_l = np.ascontiguousarray(masks.transpose(1, 0, 2))  # [P, QB//P, QB]

    in_maps = []
    for c in range(NCORES):
        sl = slice(c * HDC, (c + 1) * HDC)
        xres = np.ascontiguousarray(np.concatenate(
            [xf[:, c * ASH:(c + 1) * ASH],
             xf[:, T + c * ASH:T + (c + 1) * ASH]], axis=1))
        # [DIM, HDC] -> [P, DCH, HDC] pre-laid (contiguous per partition)
        wq_l = np.ascontiguousarray(
            wq_e[sl, :].T.reshape(DCH, P, HDC).transpose(1, 0, 2)
        ).astype(E4NP)
        wk_l = np.ascontiguousarray(
            wk_e[sl, :].T.reshape(DCH, P, HDC).transpose(1, 0, 2)
        ).astype(E4NP)
        wv_l = np.ascontiguousarray(
            wv_e[sl, :].T.reshape(DCH, P, HDC).transpose(1, 0, 2)
        ).astype(E4NP)
        in_maps.append({
            "x8T": x8,
            "rscT": rscT,
            "rscP": rscP,
            "xresT": xres,
            "wq_l": wq_l,
            "wk_l": wk_l,
            "wv_l": wv_l,
            "wo_s": wo_s,
            "w1_s": w1_s,
            "w2_s": w2_s,
            "mask_l": mask_l,
        })
    return in_maps


def _assemble(results, dtype):
    out = np.empty((B, T, DIM), dtype=np.float32)
    for c in range(NCORES):
        o = results[c]["outT"]  # [DIM, TPC] transposed
        on = o.T  # [TPC, DIM]
        out[0, c * ASH:(c + 1) * ASH, :] = on[:ASH]
        out[1, c * ASH:(c + 1) * ASH, :] = on[ASH:]
    return out.astype(dtype, copy=False)


def kernel(x, attn_norm_w, wq, wk, wv, wo, ff_norm_w, w1, w2):
    from concourse.bass_utils import run_bass_kernel_spmd

    x = np.asarray(x)
    if "nc" not in _CACHE:
        _CACHE["nc"] = _build_program()
    nc = _CACHE["nc"]

    in_maps = _host_prep(np.asarray(x, dtype=np.float32),
                         np.asarray(attn_norm_w), np.asarray(wq),
                         np.asarray(wk), np.asarray(wv), np.asarray(wo),
                         np.asarray(ff_norm_w), np.asarray(w1),
                         np.asarray(w2))
    res = run_bass_kernel_spmd(nc, in_maps, core_ids=list(range(NCORES)))
    return _assemble(res.results, x.dtype)
